# revision 14
# baseline (speedup 1.0000x reference)
"""ODE-RNN decoder kernel for Trainium2 (8 NeuronCores, data-parallel).

Math per scan step (t = 0..98), per trajectory:
    y_ode = y + (tanh(y @ Wo1 + bo1) @ Wo2 + bo2) * dt_t
    z     = sigmoid(tanh([y_ode;x] @ Wz1 + bz1) @ Wz2 + bz2)
    r     = sigmoid(tanh([y_ode;x] @ Wr1 + br1) @ Wr2 + br2)
    h     = tanh(tanh([r*y_ode;x] @ Wh1 + bh1) @ Wh2 + bh2)
    y     = (1-z)*h + z*y_ode

Layout: feature-major on-chip ([feature, batch]); batch 8192 sharded 8 ways
data-parallel (1024/core, weights replicated), CH=2 chunks of 512 columns.

The step is latency-bound by the recurrence's serial chain, so the kernel
minimizes that cycle rather than engine throughput:

- State is kept SPLIT as y_t = h_t + m_t (h = gate output, m = z*q); y is
  never materialized inside the loop.
- The ODE update is folded into the gate layer-1 GEMMs algebraically:
      Wz1y^T @ y_ode = Wz1f^T@[h;x;1] + Wz1y^T@m + (dt*[Wo2;bo2]@Wz1y)^T@[tode;1]
  so the gate path needs only tode (the ODE tanh), never y_ode. The exact
  y_ode is still computed on the side (q = m + dt*p2; y_ode = h + q) for the
  elementwise r*y_ode and m' = z*q, but those sit OFF the critical cycle.
- Every bias rides a ones-row in a moving operand; activations carry no bias
  so the z/r layer-1 tanh fuses into ONE instruction over a 2-bank PSUM tile.
- z/r layer-2 stack on partitions 0:64 (z) / 64:128 (r) of one PSUM bank so
  ONE sigmoid covers both; f32r matmuls cannot write PSUM base 64, so both
  layer-2 weights and their moving tanh tile are bf16.
- Partition-base rules (verified on hw): tensor-op INPUTS must share a
  partition base, outputs are free. y_ode is written at base 64 to pair with
  r (sigmoid rows 64:128); q/m/h stay at base 0 to pair with z.

Critical cycle per step: a5(h) -> m1 -> a1(tode) -> Ez-matmul -> a2 -> l2
-> a3(sigmoid) -> v2(r*yode) -> m7 -> a4 -> m8 -> a5.
"""

import os
import sys

sys.path.insert(0, "/opt/trn_rl_repo")

from contextlib import ExitStack

import numpy as np

import concourse.bass as bass
import concourse.tile as tile
from concourse import bacc, mybir
from concourse.bass_utils import run_bass_kernel_spmd

N_TRAJ, T, DD, DL, NU = 8192, 100, 32, 64, 100
NSTEP = T - 1
NCORES = 8
B = N_TRAJ // NCORES  # 1024 per core
CH = int(os.environ.get("KCH", "2"))    # chunks in flight per core
NCH = B // CH                           # columns per chunk
assert NCH % 8 == 0 and NCH >= 256      # f32r matmul free-dim rules

F32 = mybir.dt.float32
F32R = mybir.dt.float32r
BF16 = mybir.dt.bfloat16
TANH = mybir.ActivationFunctionType.Tanh
SIG = mybir.ActivationFunctionType.Sigmoid
ADD = mybir.AluOpType.add
MULT = mybir.AluOpType.mult


def _build():
    nc = bacc.Bacc("TRN2", target_bir_lowering=False, debug=False)

    def din(name, shape, dt=F32R):
        return nc.dram_tensor(name, list(shape), dt, kind="ExternalInput")

    K1 = DL + DD + 1  # 97: [h; x; 1]

    xs = din("xs", [NSTEP, DD + 1, B])     # host: data[:,1:,:].T + ones row
    prior = din("prior", [DL, B])
    wo1f = din("wo1f", [K1, NU])           # [Wo1; 0; bo1]
    wo1y = din("wo1y", [DL, NU])           # Wo1
    wo2b = din("wo2b", [NU + 1, DL])       # [Wo2; bo2]
    wz1f = din("wz1f", [K1, NU])           # [Wz1; bz1]
    wz1y = din("wz1y", [DL, NU])           # Wz1[:64]
    ezb0 = din("ezb0", [NU + 1, NU])       # dt0 * [Wo2;bo2] @ Wz1y
    ezb = din("ezb", [NU + 1, NU])         # dtr * [Wo2;bo2] @ Wz1y
    wr1f = din("wr1f", [K1, NU])
    wr1y = din("wr1y", [DL, NU])
    erb0 = din("erb0", [NU + 1, NU])
    erb = din("erb", [NU + 1, NU])
    wh1f = din("wh1f", [K1, NU])
    wz2b = din("wz2b", [NU + 1, DL], BF16)  # [Wz2; bz2]
    wr2b = din("wr2b", [NU + 1, DL], BF16)  # [Wr2; br2]
    wh2b = din("wh2b", [NU + 1, DL])        # [Wh2; bh2]
    dts = din("dts", [DL, NSTEP], F32)      # exact per-step dt (q path)
    zeros = din("zeros", [DL, B])           # m_0 = 0
    ones = din("ones", [1, B])              # f32r ones rows
    ones16 = din("ones16", [1, 2 * B], BF16)
    yout = nc.dram_tensor("yout", [DL, B], F32R, kind="ExternalOutput")

    mmul = nc.tensor.matmul

    with tile.TileContext(nc) as tc, ExitStack() as ctx:
        singles = ctx.enter_context(tc.tile_pool(name="singles", bufs=1))
        psum = ctx.enter_context(tc.tile_pool(name="psum", bufs=2, space="PSUM"))

        def load(dr, shape, dt=F32R):
            t_ = singles.tile(shape, dt, tag=dr.name, name="s_" + dr.name)
            nc.sync.dma_start(t_[:], dr.ap())
            return t_

        s_wo1f = load(wo1f, [K1, NU])
        s_wo1y = load(wo1y, [DL, NU])
        s_wo2b = load(wo2b, [NU + 1, DL])
        s_wz1f = load(wz1f, [K1, NU])
        s_wz1y = load(wz1y, [DL, NU])
        s_ezb0 = load(ezb0, [NU + 1, NU])
        s_ezb = load(ezb, [NU + 1, NU])
        s_wr1f = load(wr1f, [K1, NU])
        s_wr1y = load(wr1y, [DL, NU])
        s_erb0 = load(erb0, [NU + 1, NU])
        s_erb = load(erb, [NU + 1, NU])
        s_wh1f = load(wh1f, [K1, NU])
        s_wz2b = load(wz2b, [NU + 1, DL], BF16)
        s_wr2b = load(wr2b, [NU + 1, DL], BF16)
        s_wh2b = load(wh2b, [NU + 1, DL])
        s_dts = load(dts, [DL, NSTEP], F32)

        # per-chunk persistent state tiles
        st = {}
        for c in range(CH):
            cs = slice(c * NCH, (c + 1) * NCH)
            # double-buffered [h; x; 1]: step t reads hx[t%2]; a5/DMA of step
            # t write h_{t+1}/x_{t+1} into hx[(t+1)%2]
            hx = []
            for j in range(2):
                hxj = singles.tile([K1, NCH], F32R, tag=f"hx{c}_{j}",
                                   name=f"hx{c}_{j}")
                hx.append(hxj)
            nc.sync.dma_start(hx[0][0:DL, :], prior.ap()[:, cs])
            nc.sync.dma_start(hx[0][DL:K1, :], xs.ap()[0, :, cs])
            rx = singles.tile([K1, NCH], F32R, tag=f"rx{c}", name=f"rx{c}")
            m = singles.tile([DL, NCH], F32R, tag=f"m{c}", name=f"m{c}")
            nc.sync.dma_start(m[:], zeros.ap()[:, cs])
            tode = singles.tile([NU + 1, NCH], F32R, tag=f"to{c}", name=f"to{c}")
            nc.sync.dma_start(tode[NU:NU + 1, :], ones.ap()[:, 0:NCH])
            tzr = singles.tile([NU + 1, 2 * NCH], BF16, tag=f"tzr{c}",
                               name=f"tzr{c}")
            nc.sync.dma_start(tzr[NU:NU + 1, :], ones16.ap()[:, 0:2 * NCH])
            th = singles.tile([NU + 1, NCH], F32R, tag=f"th{c}", name=f"th{c}")
            nc.sync.dma_start(th[NU:NU + 1, :], ones.ap()[:, 0:NCH])
            st[c] = dict(
                hx=hx, rx=rx, m=m, tode=tode, tzr=tzr, th=th,
                q=singles.tile([DL, NCH], F32, tag=f"q{c}", name=f"q{c}"),
                yode=singles.tile([DL, NCH], F32, tag=f"yo{c}", name=f"yo{c}"),
                d=singles.tile([2 * DL, NCH], F32, tag=f"d{c}", name=f"d{c}"),
                szr=singles.tile([2 * DL, NCH], F32, tag=f"szr{c}",
                                 name=f"szr{c}"),
            )

        def half1(c, t):
            """DMA, ODE l1+tanh, ODE l2, q, y_ode, zr layer-1 + fused tanh.
            The four pzr1 matmuls that don't need tode are emitted before a1
            so PE runs them while ACT does the tanh; ez/er follow a1."""
            s = st[c]
            cs = slice(c * NCH, (c + 1) * NCH)
            cur = s["hx"][t % 2]
            nxt = s["hx"][(t + 1) % 2]
            ez = s_ezb0 if t == 0 else s_ezb
            er = s_erb0 if t == 0 else s_erb
            if t + 1 < NSTEP:
                nc.sync.dma_start(nxt[DL:K1, :], xs.ap()[t + 1, :, cs])
            nc.sync.dma_start(s["rx"][DL:K1, :], xs.ap()[t, :, cs])
            p1 = psum.tile([NU, NCH], F32, tag="l1", name="p1", bufs=2)
            mmul(p1[:], s_wo1f[:], cur[:], start=True, stop=False)
            mmul(p1[:], s_wo1y[:], s["m"][:], start=False, stop=True)
            pzr1 = psum.tile([NU, 2 * NCH], F32, tag="l1w", name="pzr1",
                             bufs=2)
            mmul(pzr1[:, 0:NCH], s_wz1f[:], cur[:], start=True, stop=False)
            mmul(pzr1[:, 0:NCH], s_wz1y[:], s["m"][:], start=False, stop=False)
            mmul(pzr1[:, NCH:2 * NCH], s_wr1f[:], cur[:],
                 start=True, stop=False)
            mmul(pzr1[:, NCH:2 * NCH], s_wr1y[:], s["m"][:],
                 start=False, stop=False)
            nc.scalar.activation(s["tode"][0:NU, :], p1[:], TANH)
            mmul(pzr1[:, 0:NCH], ez[:], s["tode"][:], start=False, stop=True)
            mmul(pzr1[:, NCH:2 * NCH], er[:], s["tode"][:],
                 start=False, stop=True)
            p2 = psum.tile([2 * DL, NCH], F32, tag="b", name="p2", bufs=2)
            mmul(p2[0:DL, :], s_wo2b[:], s["tode"][:])
            nc.scalar.activation(s["tzr"][0:NU, :], pzr1[:], TANH)
            # q = p2*dt + m (exact per-step dt); y_ode = h + q — both off the
            # critical cycle (needed by v2/d-sub later in half2)
            nc.vector.scalar_tensor_tensor(
                s["q"][:], p2[0:DL, :], s_dts[:, t:t + 1],
                s["m"][:].bitcast(F32), op0=MULT, op1=ADD)
            nc.gpsimd.tensor_add(s["yode"][:], cur[0:DL, :].bitcast(F32),
                                 s["q"][:])

        def half2(c, t):
            """zr layer-2 + sigmoid, h gate, GRU combine tail."""
            s = st[c]
            nxt = s["hx"][(t + 1) % 2]
            pzr2 = psum.tile([2 * DL, NCH], F32, tag="b", name="pzr2", bufs=2)
            mmul(pzr2[0:DL, :], s_wr2b[:], s["tzr"][:, NCH:2 * NCH])
            mmul(pzr2[DL:2 * DL, :], s_wz2b[:], s["tzr"][:, 0:NCH])
            # rows 0:64 = r, 64:128 = z
            nc.scalar.activation(s["szr"][:], pzr2[:], SIG)
            # r*y_ode (all base 0)
            nc.vector.tensor_mul(s["rx"][0:DL, :], s["szr"][0:DL, :],
                                 s["yode"][:])
            ph = psum.tile([NU, NCH], F32, tag="l1", name="ph", bufs=2)
            mmul(ph[:], s_wh1f[:], s["rx"][:])
            nc.scalar.activation(s["th"][0:NU, :], ph[:], TANH)
            ph2 = psum.tile([2 * DL, NCH], F32, tag="b", name="ph2", bufs=2)
            mmul(ph2[0:DL, :], s_wh2b[:], s["th"][:])
            nc.scalar.activation(nxt[0:DL, :], ph2[0:DL, :], TANH)
            # m' = z*(y_ode - h_new): d at base 64 so the z-mul's inputs
            # share a base (z sits in sigmoid rows 64:128)
            nc.vector.tensor_sub(s["d"][DL:2 * DL, :], s["yode"][:],
                                 nxt[0:DL, :].bitcast(F32))
            nc.vector.tensor_mul(s["m"][:], s["szr"][DL:2 * DL, :],
                                 s["d"][DL:2 * DL, :])

        # Anti-phased software pipeline: chunk 1 runs half a step behind
        # chunk 0 so their ACT/DVE/PE bursts interleave instead of colliding.
        assert CH == 2
        for t in range(NSTEP):
            half1(0, t)
            if t > 0:
                half2(1, t - 1)
            half2(0, t)
            half1(1, t)
        half2(1, NSTEP - 1)

        # y_final = h_99 + m_99
        for c in range(CH):
            cs = slice(c * NCH, (c + 1) * NCH)
            s = st[c]
            yfin = singles.tile([DL, NCH], F32R, tag=f"yf{c}", name=f"yf{c}")
            nc.vector.tensor_add(yfin[:], s["hx"][NSTEP % 2][0:DL, :].bitcast(F32),
                                 s["m"][:].bitcast(F32))
            nc.sync.dma_start(yout.ap()[:, cs], yfin[:])

    nc.compile()
    return nc


_NC_CACHE = None


def _get_nc():
    global _NC_CACHE
    if _NC_CACHE is None:
        _NC_CACHE = _build()
    return _NC_CACHE


def _prep_core_inputs(data, time_steps, prior, weights):
    """Host-side glue: shard + transpose into the kernel's layouts."""
    import ml_dtypes
    dts = np.concatenate([time_steps[1:2] - time_steps[0:1],
                          time_steps[:-2] - time_steps[1:-1]]).astype(np.float32)
    dts_b = np.ascontiguousarray(
        np.broadcast_to(dts[None, :], (DL, NSTEP))).astype(np.float32)
    (Wo1, bo1, Wo2, bo2, Wz1, bz1, Wz2, bz2,
     Wr1, br1, Wr2, br2, Wh1, bh1, Wh2, bh2) = weights

    def wb(W, b):
        return np.concatenate([W, b[None, :]], axis=0)

    wo2b = wb(Wo2, bo2)                       # [101, 64]
    dt0 = float(dts[0])
    dtr = float(dts[1]) if NSTEP > 1 else dt0
    shared = {
        "wo1f": np.concatenate(
            [Wo1, np.zeros((DD, NU), np.float32), bo1[None, :]], axis=0),
        "wo1y": Wo1,
        "wo2b": wo2b,
        "wz1f": wb(Wz1, bz1), "wz1y": Wz1[:DL],
        "ezb0": dt0 * (wo2b @ Wz1[:DL]), "ezb": dtr * (wo2b @ Wz1[:DL]),
        "wr1f": wb(Wr1, br1), "wr1y": Wr1[:DL],
        "erb0": dt0 * (wo2b @ Wr1[:DL]), "erb": dtr * (wo2b @ Wr1[:DL]),
        "wh1f": wb(Wh1, bh1),
        "wh2b": wb(Wh2, bh2),
        "dts": dts_b,
        "zeros": np.zeros((DL, B), np.float32),
        "ones": np.ones((1, B), np.float32),
    }
    shared = {k: np.ascontiguousarray(v, dtype=np.float32)
              for k, v in shared.items()}
    shared["wz2b"] = wb(Wz2, bz2).astype(ml_dtypes.bfloat16)
    shared["wr2b"] = wb(Wr2, br2).astype(ml_dtypes.bfloat16)
    shared["ones16"] = np.ones((1, 2 * B), ml_dtypes.bfloat16)
    in_maps = []
    ones_row = np.ones((1, B), np.float32)
    for i in range(NCORES):
        ts_ = slice(i * B, (i + 1) * B)
        xt = data[ts_, 1:, :].transpose(1, 2, 0)  # [NSTEP, DD, B]
        xs1 = np.concatenate(
            [xt, np.broadcast_to(ones_row, (NSTEP, 1, B))], axis=1)
        xs1 = np.ascontiguousarray(xs1).astype(np.float32)
        pr = np.ascontiguousarray(prior[ts_].T).astype(np.float32)
        in_maps.append({"xs": xs1, "prior": pr, **shared})
    return in_maps


def kernel(data, time_steps, prior,
           Wo1, bo1, Wo2, bo2,
           Wz1, bz1, Wz2, bz2,
           Wr1, br1, Wr2, br2,
           Wh1, bh1, Wh2, bh2):
    data = np.asarray(data, dtype=np.float32)
    time_steps = np.asarray(time_steps, dtype=np.float32)
    prior = np.asarray(prior, dtype=np.float32)
    weights = [np.asarray(w, dtype=np.float32) for w in
               (Wo1, bo1, Wo2, bo2, Wz1, bz1, Wz2, bz2,
                Wr1, br1, Wr2, br2, Wh1, bh1, Wh2, bh2)]
    nc = _get_nc()
    in_maps = _prep_core_inputs(data, time_steps, prior, weights)
    res = run_bass_kernel_spmd(nc, in_maps, core_ids=list(range(NCORES)))
    out = np.empty((N_TRAJ, DL), dtype=np.float32)
    for i in range(NCORES):
        out[i * B:(i + 1) * B] = res.results[i]["yout"].T
    return out
